# revision 12
# baseline (speedup 1.0000x reference)
"""MultiHeadAttention Trainium2 kernel (8 NeuronCores, SPMD, no collectives).

Reference model: B=4, S=2048, D=1024, H=16, Dh=64.
  q/k/v = split_heads(x @ W.T + b); scores = q k^T / sqrt(Dh); mask==0 -> -1e9;
  softmax; out = (attn v) @ fc_w.T + fc_b.

Sharding (tensor-parallel over heads): core c handles batch b=c//2 and head
group g=c%2 (8 of 16 heads) for the full 2048-query range. Every projection
is done exactly once per core (q/k/v project 2048 tokens into this group's
512 output dims). The fc output is a PARTIAL product over this group's 512
input dims; the host sums the two partials per batch and adds fc_b.

v3: TILED SCORE MATMULS. The PE array row-tiles: two K=64 matmuls at row
positions 0 and 64 execute overlapped (measured 111ns/instr vs 216ns for
K=128, while K=64 at a FIXED position runs at 425ns from the HAM clock
gate). Attention therefore processes HEAD PAIRS (2j, 2j+1): their K/Q live
in the two partition halves of KT/QT[:, j], so the per-head score matmuls
naturally alternate positions. Q needs no zero-padding (QTZ gone). Score
PE time halves: ~109us -> ~55us.

PSUM budget (exactly 8 banks): scores sc_e+sc_o [P,1024] f32 x2 = 4 banks,
single-buffered per head (the WAR on exp is covered by the 2-round PV lag);
PV 4x[P,512] = 4 banks for ONE head pair. The pair's psums are EVACUATED to
SBUF right after their last accumulation (frees banks for the next pair),
and the softmax normalization runs from SBUF: out = pv*(1/rowsum) + bv
(per-partition bias add replaces the old rank-1 matmul; rowsum comes from
the ones-column row 64 of the evacuated tile).

Other structure as v2:
  - x arrives host-transposed; K^T/Q^T [d_out_local, t] from projection
    matmuls with ScalarE bias evacuations; V in [t, d_out] layout with a
    ones column per head (66-wide groups).
  - exp on ScalarE straight out of PSUM (scale 0.125 folded); mask applied
    multiplicatively on VectorE; PE round order interleaves K=64 score
    matmuls with K=128 PV matmuls so the clock gate never sees two
    same-position K=64s back to back.
  - mask SBUF holds only the current query half; the other half re-DMAs
    per-chunk while the last head pair of the half drains.
"""

import os

import numpy as np
import ml_dtypes

BF16 = ml_dtypes.bfloat16

D = 1024
S = 2048
B = 4
HL = 8       # local heads per core
DL = 512     # local d_out per core
TQ = 1024    # queries per (head, q-half) block
P = 128
N_CORES = 8

_CACHED = {}


def _build():
    import concourse.bass as bass
    import concourse.mybir as mybir
    import concourse.tile as tile
    from concourse import bacc

    BF = mybir.dt.bfloat16
    F32 = mybir.dt.float32
    AF = mybir.ActivationFunctionType

    nc = bacc.Bacc("TRN2", target_bir_lowering=False, debug=False)

    xqT = nc.dram_tensor("xqT", [D, S], BF, kind="ExternalInput").ap()
    xkT = nc.dram_tensor("xkT", [D, S], BF, kind="ExternalInput").ap()
    xvT = nc.dram_tensor("xvT", [D, S], BF, kind="ExternalInput").ap()
    wqT = nc.dram_tensor("wqT", [D, DL], BF, kind="ExternalInput").ap()
    wkT = nc.dram_tensor("wkT", [D, DL], BF, kind="ExternalInput").ap()
    wvT = nc.dram_tensor("wvT", [D, DL], BF, kind="ExternalInput").ap()
    fcT = nc.dram_tensor("fcT", [DL, D], BF, kind="ExternalInput").ap()
    maskT = nc.dram_tensor("maskT", [S, S], BF, kind="ExternalInput").ap()
    bq_d = nc.dram_tensor("bq", [P, 4], F32, kind="ExternalInput").ap()
    bk_d = nc.dram_tensor("bk", [P, 4], F32, kind="ExternalInput").ap()
    bv_d = nc.dram_tensor("bv", [P, HL], F32, kind="ExternalInput").ap()
    out = nc.dram_tensor("out", [S, D], F32, kind="ExternalOutput").ap()

    VGW = 66  # per-head group width in V storage: 64 V cols + ones col + pad
    VGPAD = 62  # tail pad so every head can present a 128-col lhsT

    msk_r = maskT.rearrange("(t p) q -> p t q", p=P)

    with tile.TileContext(nc) as tc:
        with tc.tile_pool(name="const", bufs=1) as const:
            QT = const.tile([P, 4, S], BF)        # Q^T  (d_out_local, q)
            KT = const.tile([P, 4, S], BF)        # K^T  (d_out_local, tk)
            VG = const.tile([P, 16, HL * VGW + VGPAD], BF)  # V (+ones)
            AOT = const.tile([P, 4, S], BF)       # attn-out^T
            MSK = const.tile([P, 16, TQ], BF)     # mask^T for CURRENT q half
            bq_s = const.tile([P, 4], F32)
            bk_s = const.tile([P, 4], F32)
            # head bias replicated across both partition halves so the
            # per-partition scalar operand can be sliced base-aligned with
            # either AOT half
            bv_s = const.tile([P, HL], F32)
            ones_bf = const.tile([1, P], BF)

            nc.scalar.dma_start(bq_s[:], bq_d)
            nc.scalar.dma_start(bk_s[:], bk_d)
            nc.scalar.dma_start(bv_s[:], bv_d)
            nc.vector.memset(ones_bf[:], 1.0)
            # touch partition_broadcast once now: the first gpsimd custom
            # instruction pays a multi-us program-load cost
            gpswarm = const.tile([64, P], BF)
            nc.gpsimd.partition_broadcast(gpswarm[:], ones_bf[:])
            # V group ones columns (col 64) + junk col 65 + tail pad
            for tt in range(16):
                nc.vector.memset(
                    VG[:, tt, 0:HL * VGW].rearrange(
                        "p (h c) -> p h c", c=VGW)[:, :, 64:66],
                    1.0,
                )
                nc.vector.memset(VG[:, tt, HL * VGW:], 1.0)
            # mask for q-half 0: lands during V/K projections
            nc.scalar.dma_start(MSK[:, 0:8], msk_r[:, 0:8, 0:TQ])
            nc.scalar.dma_start(MSK[:, 8:16], msk_r[:, 8:16, 0:TQ])

            # ---------------- projections ----------------
            with (
                tc.tile_pool(name="xin", bufs=16) as xin,
                tc.tile_pool(name="wpool", bufs=2) as wpool,
                tc.tile_pool(name="ppsum", bufs=3, space="PSUM") as ppsum,
            ):
                # V projection: V[t, do] = sum_di xvT[di,t] * wvT[di,do]
                wv_s = wpool.tile([P, 8, DL], BF, tag="w")
                wv_r = wvT.rearrange("(j p) n -> p j n", p=P)
                nc.sync.dma_start(wv_s[:, 0:4], wv_r[:, 0:4])
                xv = [[None, None] for _ in range(8)]
                for di in range(4):
                    t_ = xin.tile([P, TQ], BF, tag="xt", name=f"xv{di}_0")
                    nc.sync.dma_start(t_[:], xvT[di * P:(di + 1) * P, 0:TQ])
                    xv[di][0] = t_
                nc.sync.dma_start(wv_s[:, 4:8], wv_r[:, 4:8])
                for di in range(4, 8):
                    t_ = xin.tile([P, TQ], BF, tag="xt", name=f"xv{di}_0")
                    nc.sync.dma_start(t_[:], xvT[di * P:(di + 1) * P, 0:TQ])
                    xv[di][0] = t_
                for di in range(8):
                    t_ = xin.tile([P, TQ], BF, tag="xt", name=f"xv{di}_1")
                    nc.sync.dma_start(t_[:], xvT[di * P:(di + 1) * P, TQ:S])
                    xv[di][1] = t_
                for tt in range(16):
                    ps = ppsum.tile([P, DL], F32, tag="pp")
                    for di in range(8):
                        nc.tensor.matmul(
                            ps[:],
                            lhsT=xv[di][tt // 8][
                                :, (tt % 8) * P:(tt % 8 + 1) * P],
                            rhs=wv_s[:, di, :],
                            start=(di == 0),
                            stop=(di == 7),
                        )
                    dst = VG[:, tt, 0:HL * VGW].rearrange(
                        "p (h c) -> p h c", c=VGW
                    )[:, :, 0:64]
                    srcp = ps.rearrange("p (h c) -> p h c", c=64)
                    if tt % 2 == 0:
                        nc.vector.tensor_copy(dst, srcp)
                    else:
                        nc.scalar.copy(dst, srcp)

                # K projection: K^T[do, tk] = sum_di wkT[di,do] * xkT[di,tk]
                wk_s = wpool.tile([P, 8, DL], BF, tag="w")
                wk_r = wkT.rearrange("(j p) n -> p j n", p=P)
                nc.sync.dma_start(wk_s[:, 0:4], wk_r[:, 0:4])
                nc.sync.dma_start(wk_s[:, 4:8], wk_r[:, 4:8])
                xk = [[None, None] for _ in range(8)]
                for hf in range(2):
                    for di in range(8):
                        t_ = xin.tile([P, TQ], BF, tag="xt", name=f"xk{di}_{hf}")
                        nc.sync.dma_start(
                            t_[:], xkT[di * P:(di + 1) * P,
                                       hf * TQ:(hf + 1) * TQ]
                        )
                        xk[di][hf] = t_
                for j in range(4):
                    for n in range(4):
                        ps = ppsum.tile([P, DL], F32, tag="pp")
                        for di in range(8):
                            nc.tensor.matmul(
                                ps[:],
                                lhsT=wk_s[:, di, j * P:(j + 1) * P],
                                rhs=xk[di][n // 2][
                                    :, (n % 2) * DL:(n % 2 + 1) * DL],
                                start=(di == 0),
                                stop=(di == 7),
                            )
                        nc.scalar.activation(
                            KT[:, j, n * DL:(n + 1) * DL], ps[:],
                            AF.Identity, bias=bk_s[:, j:j + 1],
                        )

                # Q projection
                wq_s = wpool.tile([P, 8, DL], BF, tag="w")
                wq_r = wqT.rearrange("(j p) n -> p j n", p=P)
                nc.sync.dma_start(wq_s[:, 0:4], wq_r[:, 0:4])
                nc.sync.dma_start(wq_s[:, 4:8], wq_r[:, 4:8])
                xq = [[None, None] for _ in range(8)]
                for hf in range(2):
                    for di in range(8):
                        t_ = xin.tile([P, TQ], BF, tag="xt", name=f"xq{di}_{hf}")
                        nc.sync.dma_start(
                            t_[:], xqT[di * P:(di + 1) * P,
                                       hf * TQ:(hf + 1) * TQ]
                        )
                        xq[di][hf] = t_
                for hf in range(2):
                    for j in range(4):
                        for n2 in range(2):
                            n = hf * 2 + n2
                            ps = ppsum.tile([P, DL], F32, tag="pp")
                            for di in range(8):
                                nc.tensor.matmul(
                                    ps[:],
                                    lhsT=wq_s[:, di, j * P:(j + 1) * P],
                                    rhs=xq[di][hf][
                                        :, n2 * DL:(n2 + 1) * DL],
                                    start=(di == 0),
                                    stop=(di == 7),
                                )
                            nc.scalar.activation(
                                QT[:, j, n * DL:(n + 1) * DL], ps[:],
                                AF.Identity, bias=bq_s[:, j:j + 1],
                            )

            # opool opens before attention so the FCT DMA lands mid-attention
            with tc.tile_pool(name="opool", bufs=2) as opool:
                FCT = opool.tile([P, 4, D], BF, tag="fct")
                nc.scalar.dma_start(
                    FCT[:], fcT.rearrange("(j p) n -> p j n", p=P))

                # ---------------- attention ----------------
                # Head-pair rounds: per (qh, jp, tk), the pair's score
                # matmuls (K=64, row positions 0/64) interleave with the
                # K=128 PV matmuls of the round two back.
                with (
                    tc.tile_pool(name="spsum", bufs=2, space="PSUM") as spsum,
                    tc.tile_pool(name="vpsum", bufs=4, space="PSUM") as vpsum,
                    tc.tile_pool(name="ppool", bufs=6) as ppool,
                    tc.tile_pool(name="npool", bufs=2) as npool,
                ):
                    pvq = []        # queue of pending PV matmul closures
                    norm_q = []     # queue of staged norm-step closures

                    def pop_pv(drain=False):
                        # keep 4 closures (one round) queued: a 2-round lag
                        # between scores and PV. This also guarantees the
                        # prev pair's tk=15 PVs all pop by round tk=1 of the
                        # next pair, BEFORE the evac reads emitted there.
                        if len(pvq) > 4 or (drain and pvq):
                            pvq.pop(0)()

                    def pop_norm():
                        if norm_q:
                            norm_q.pop(0)()

                    def emit_round(qh, jp, tk, pv_tiles, prev):
                        q0 = qh * TQ
                        sc_e = spsum.tile([P, TQ], F32, tag="sc",
                                          name=f"sce{qh}_{jp}_{tk}")
                        sc_o = spsum.tile([P, TQ], F32, tag="sc",
                                          name=f"sco{qh}_{jp}_{tk}")
                        # PE order: K=64 score matmuls alternate row position
                        # (e at 0, o at 64) and are separated by K=128 PV
                        # matmuls so the clock gate stays at full width.
                        nc.tensor.matmul(
                            sc_e[:, 0:512],
                            lhsT=KT[0:64, jp, tk * P:(tk + 1) * P],
                            rhs=QT[0:64, jp, q0:q0 + 512],
                            start=True, stop=True,
                        )
                        pop_pv()
                        nc.tensor.matmul(
                            sc_o[:, 0:512],
                            lhsT=KT[64:P, jp, tk * P:(tk + 1) * P],
                            rhs=QT[64:P, jp, q0:q0 + 512],
                            start=True, stop=True,
                        )
                        pop_pv()
                        nc.tensor.matmul(
                            sc_e[:, 512:TQ],
                            lhsT=KT[0:64, jp, tk * P:(tk + 1) * P],
                            rhs=QT[0:64, jp, q0 + 512:q0 + TQ],
                            start=True, stop=True,
                        )
                        pop_pv()
                        nc.tensor.matmul(
                            sc_o[:, 512:TQ],
                            lhsT=KT[64:P, jp, tk * P:(tk + 1) * P],
                            rhs=QT[64:P, jp, q0 + 512:q0 + TQ],
                            start=True, stop=True,
                        )
                        pop_pv()
                        pt_e = ppool.tile([P, TQ], BF, tag="pt",
                                          name=f"pte{qh}_{jp}_{tk}")
                        pt_o = ppool.tile([P, TQ], BF, tag="pt",
                                          name=f"pto{qh}_{jp}_{tk}")
                        nc.scalar.activation(pt_e[:], sc_e[:], AF.Exp,
                                             scale=0.125)
                        nc.vector.tensor_mul(pt_e[:], pt_e[:], MSK[:, tk])
                        nc.scalar.activation(pt_o[:], sc_o[:], AF.Exp,
                                             scale=0.125)
                        nc.vector.tensor_mul(pt_o[:], pt_o[:], MSK[:, tk])
                        # queue this round's PV matmuls (run 2 rounds later)
                        for hi, pt in ((0, pt_e), (1, pt_o)):
                            h = 2 * jp + hi
                            for n in range(2):
                                def pv_mm(h=h, hi=hi, n=n, pt=pt, tk=tk):
                                    nc.tensor.matmul(
                                        pv_tiles[2 * hi + n][:],
                                        lhsT=VG[:, tk,
                                                h * VGW:h * VGW + P],
                                        rhs=pt[:, n * 512:(n + 1) * 512],
                                        start=(tk == 0),
                                        stop=(tk == 15),
                                    )
                                pvq.append(pv_mm)
                        # staged norm steps for the PREVIOUS pair's chain.
                        # Both evacs must come right after round tk==1 (the
                        # round whose pops drain the prev pair's tk=15 PV
                        # matmuls) and before round tk==2's pops write the
                        # current pair's (same-buffer) psum tiles.
                        if tk == 1:
                            pop_norm()
                            pop_norm()
                        elif tk in (2, 3):
                            pop_norm()

                    def queue_norm(qh, jp, pv_tiles):
                        # evac psums -> SBUF, then normalize + bias from SBUF
                        q0 = qh * TQ
                        pvs = [None, None]

                        def evac(hi):
                            def f(hi=hi):
                                t = npool.tile([65, TQ], F32, tag=f"pvs{hi}",
                                               bufs=1,
                                               name=f"pvs{qh}_{jp}_{hi}")
                                pvs[hi] = t
                                for n in range(2):
                                    nc.vector.tensor_copy(
                                        t[:, n * 512:(n + 1) * 512],
                                        pv_tiles[2 * hi + n][0:65, :])
                            return f

                        def norm(hi):
                            def f(hi=hi):
                                h = 2 * jp + hi
                                bp = 64 * hi
                                rs = npool.tile([1, TQ], F32, tag="rs",
                                                name=f"rs{qh}_{jp}_{hi}")
                                nc.vector.tensor_copy(
                                    rs[:], pvs[hi][64:65, :])
                                rc = npool.tile([1, TQ], F32, tag="rc",
                                                name=f"rc{qh}_{jp}_{hi}")
                                nc.vector.reciprocal_approx_fast(
                                    rc[:], rs[:])
                                bcs = npool.tile([64, TQ], F32, tag="bcs",
                                                 name=f"bcs{qh}_{jp}_{hi}")
                                nc.gpsimd.partition_broadcast(bcs[:], rc[:])
                                dst = AOT[bp:bp + 64, jp, q0:q0 + TQ]
                                nc.vector.tensor_mul(
                                    dst, pvs[hi][0:64, :], bcs[:])
                                nc.vector.tensor_scalar_add(
                                    dst, dst, bv_s[bp:bp + 64, h:h + 1])
                            return f

                        norm_q.append(evac(0))
                        norm_q.append(evac(1))
                        norm_q.append(norm(0))
                        norm_q.append(norm(1))

                    for qh in range(2):
                        for jp in range(4):
                            pv_tiles = [
                                vpsum.tile([P, 512], F32, tag="pv",
                                           name=f"pv{qh}_{jp}_{k}")
                                for k in range(4)
                            ]
                            for tk in range(16):
                                emit_round(qh, jp, tk, pv_tiles, prev=None)
                            queue_norm(qh, jp, pv_tiles)
                            if qh == 0 and jp == 3:
                                # refill mask with q-half 1 while it drains
                                for tk in range(16):
                                    nc.sync.dma_start(
                                        MSK[:, tk], msk_r[:, tk, TQ:S])
                    # drain: 8 pending PV matmuls + last norm chain, with
                    # full-array warm keepers against the HAM clock monitor
                    while pvq:
                        pop_pv(drain=True)
                    warm_sc = spsum.tile([P, TQ], F32, tag="sc",
                                         name="warmsc")
                    wi = 0
                    while norm_q:
                        for w in range(3):
                            nc.tensor.matmul(
                                warm_sc[:, (wi % 2) * 512:(wi % 2 + 1) * 512],
                                lhsT=KT[:, 0, 0:P],
                                rhs=QT[:, 0, 0:512],
                                start=True, stop=True,
                                skip_group_check=True,
                            )
                            wi += 1
                        pop_norm()

                # ---------------- output projection (partial) ----------------
                with tc.tile_pool(name="fpsum", bufs=4, space="PSUM") as fpsum:
                    warm_fp = fpsum.tile([P, 512], F32, tag="fp", name="warmfp")
                    for w in range(8):
                        nc.tensor.matmul(
                            warm_fp[:],
                            lhsT=AOT[:, 0, 0:P],
                            rhs=AOT[:, 0, 0:512],
                            start=True, stop=True,
                            skip_group_check=True,
                        )
                    for tt in range(16):
                        ob = opool.tile([P, D], F32, tag="ob")
                        for n in range(2):
                            ps = fpsum.tile([P, 512], F32, tag="fp")
                            for j in range(4):
                                nc.tensor.matmul(
                                    ps[:],
                                    lhsT=AOT[:, j, tt * P:(tt + 1) * P],
                                    rhs=FCT[:, j, n * 512:(n + 1) * 512],
                                    start=(j == 0),
                                    stop=(j == 3),
                                )
                            if (tt * 2 + n) % 2 == 0:
                                nc.vector.tensor_copy(
                                    ob[:, n * 512:(n + 1) * 512], ps[:])
                            else:
                                nc.scalar.copy(
                                    ob[:, n * 512:(n + 1) * 512], ps[:])
                        nc.sync.dma_start(out[tt * P:(tt + 1) * P, :], ob[:])

    nc.compile()
    return nc


def _get_nc():
    if "nc" not in _CACHED:
        _CACHED["nc"] = _build()
    return _CACHED["nc"]


def kernel(**inputs):
    from concourse import bass_utils

    query = np.asarray(inputs["query"], np.float32)
    key_in = np.asarray(inputs["key_in"], np.float32)
    value = np.asarray(inputs["value"], np.float32)
    mask = np.asarray(inputs["mask"])
    wq_w = np.asarray(inputs["wq_w"], np.float32)
    wq_b = np.asarray(inputs["wq_b"], np.float32)
    wk_w = np.asarray(inputs["wk_w"], np.float32)
    wk_b = np.asarray(inputs["wk_b"], np.float32)
    wv_w = np.asarray(inputs["wv_w"], np.float32)
    wv_b = np.asarray(inputs["wv_b"], np.float32)
    fc_w = np.asarray(inputs["fc_w"], np.float32)
    fc_b = np.asarray(inputs["fc_b"], np.float32)

    def c(a):
        return np.ascontiguousarray(a)

    # per-head-group (tensor-parallel) weight slices
    gshard = []
    for g in range(2):
        lo, hi = g * DL, (g + 1) * DL
        gshard.append({
            "wqT": c(wq_w[lo:hi, :].T.astype(BF16)),
            "wkT": c(wk_w[lo:hi, :].T.astype(BF16)),
            "wvT": c(wv_w[lo:hi, :].T.astype(BF16)),
            "fcT": c(fc_w[:, lo:hi].T.astype(BF16)),
            "bq": c(wq_b[lo:hi].reshape(4, P).T.astype(np.float32)),
            "bk": c(wk_b[lo:hi].reshape(4, P).T.astype(np.float32)),
            "bv": c(np.tile(wv_b[lo:hi].reshape(HL, 64).T, (2, 1))
                    .astype(np.float32)),
        })

    # per-batch activation transposes (shared by the two cores of a pair)
    bshard = []
    for b in range(B):
        bshard.append({
            "xqT": c(query[b].T.astype(BF16)),
            "xkT": c(key_in[b].T.astype(BF16)),
            "xvT": c(value[b].T.astype(BF16)),
            "maskT": c(mask[b].T.astype(BF16)),
        })

    in_maps = []
    for core in range(N_CORES):
        b, g = core // 2, core % 2
        m = dict(gshard[g])
        m.update(bshard[b])
        in_maps.append(m)

    nc = _get_nc()
    trace = bool(int(os.environ.get("KERNEL_TRACE", "0")))
    res = bass_utils.run_bass_kernel_spmd(
        nc, in_maps, core_ids=list(range(N_CORES)), trace=trace,
        **({"trace_cores": [0]} if trace else {}),
    )
    _CACHED["last_results"] = res

    full = np.empty((B, S, D), np.float32)
    fcb = fc_b.reshape(1, D)
    for b in range(B):
        full[b] = res.results[2 * b]["out"]
        full[b] += res.results[2 * b + 1]["out"]
        full[b] += fcb
    return full
